# revision 25
# baseline (speedup 1.0000x reference)
"""AtomMotifAttention Trainium2 kernel.

Strategy (8 NeuronCores, SPMD single program):
  - Shard atoms: core c handles rows [c*1536, (c+1)*1536).
  - Both batch vectors are sorted, so the attention is block-diagonal: the
    atoms of one core only attend to a narrow contiguous window of motifs
    (max span ~235 of 1536 here).  Each core gets a host-sliced window of
    motif_x (W columns, W = max span over cores rounded to 128), computes
    scores/softmax only on that window, and assembles each 128-row tile of
    attn in a [128, 4*1536] staging tile whose columns [h*1536, h*1536+W)
    hold the window and the rest are zeros.  The full 302 MB attn tensor
    is written by the device; the host only rolls each core's block by its
    window start during unsharding (pure memcpy of device-produced bytes).
  - The batch mask is folded into the scores matmul as a one-hot bf16
    accumulation: psum = q.k/8 + 1000*eq, exp(psum - 1000) underflows to
    exactly 0 for masked entries (same as the reference, where
    exp(-1e9) == 0) and subtract-max is unnecessary because every graph
    has >=1 motif and scores are O(10).
  - attn @ v needs attn^T: PE-transpose the normalized window tiles.
  - Hot matmuls run in float32r (4x faster than float32 on TRN2 for
    moving dim >= 256; measured exact for these magnitudes).  float32r
    operands must be produced "rounded": the psum->sbuf copybacks (DVE
    tensor_copy / ACT Identity) write float32r tiles directly.
  - Q/K biases ride the ACT-Identity copyback (per-partition bias), V/O
    biases are K=1 ones-matmuls accumulated into psum.
  - Residual + LayerNorm fused on-chip; ln_g/ln_b applied on host only if
    nontrivial (they are 1/0 here; (y-mu)*rstd*g+b is exact either way).

Compile-time constraints worked around:
  - Each PE instruction supports exactly ONE fresh semaphore wait, so PE
    inputs are packed into few big DMAs and every PE-feeding producer
    keeps a PE instruction's fresh deps on a single semaphore.
  - bacc.Bacc + nc.compile() is required (raw bass.Bass BIR fails walrus
    codegen on multi-wait instructions).
"""

import os
import sys

import numpy as np

sys.path.insert(0, "/opt/trn_rl_repo")

import ml_dtypes

import concourse.bacc as bacc
import concourse.mybir as mybir
import concourse.tile as tile
from concourse.bass_utils import run_bass_kernel_spmd

F32 = mybir.dt.float32
F32R = mybir.dt.float32r
BF16 = mybir.dt.bfloat16
AF = mybir.ActivationFunctionType
ALU = mybir.AluOpType

N_CORES = 8
N_ATOMS = 12288
N_MOTIFS = 1536
D = 256
H = 4
HD = 64
NP = N_ATOMS // N_CORES      # rows per core = 1536
T = NP // 128                # 12 tiles of 128 rows per core
MASK_C = 1000.0
LN_EPS = 1e-5

# combo_x split: cx0 = tiles 0-3 (early), cx1 = tiles 4-11
CX0F = 4 * D
CX1F = (T - 4) * D


def _build_program(W):
    """Build the SPMD bass program for window width W (multiple of 128)."""
    NB = W // 128  # motif window blocks

    # combo_w layout (f32): identity, motif window, 4 weights, bias columns
    off_id = 0
    off_mw = off_id + 128
    off_wq = off_mw + NB * D
    off_wk = off_wq + 2 * D
    off_wv = off_wk + 2 * D
    off_wo = off_wv + 2 * D
    off_bq = off_wo + 2 * D         # [128, 2] columns of bq/8
    off_bk = off_bq + 2             # [128, 2] columns of bk
    cwf = off_bk + 2
    # combo16 layout (bf16): A1000 [128, NP], B [128, W]
    c16f = NP + W

    nc = bacc.Bacc(None, target_bir_lowering=False)

    cx0_d = nc.dram_tensor("combo_x0", [128, CX0F], F32,
                           kind="ExternalInput")
    cx1_d = nc.dram_tensor("combo_x1", [128, CX1F], F32,
                           kind="ExternalInput")
    cw_d = nc.dram_tensor("combo_w", [128, cwf], F32, kind="ExternalInput")
    c16_d = nc.dram_tensor("combo16", [128, c16f], BF16, kind="ExternalInput")
    # rows: [bv(256), bo(256), ones(512)] on partition 0
    rows_d = nc.dram_tensor("rows", [1, 1024], F32, kind="ExternalInput")
    attn_d = nc.dram_tensor("attn_out", [NP, H, N_MOTIFS], F32,
                            kind="ExternalOutput")
    out_d = nc.dram_tensor("out", [NP, D], F32, kind="ExternalOutput")

    with tile.TileContext(nc) as tc:
        with (
            tc.tile_pool(name="persist", bufs=1) as persist,
            tc.tile_pool(name="work", bufs=3) as work,
            tc.tile_pool(name="ps_sc", bufs=2, space="PSUM") as ps_sc,
            tc.tile_pool(name="ps_tr", bufs=2, space="PSUM") as ps_tr,
            tc.tile_pool(name="ps_ao", bufs=2, space="PSUM") as ps_ao,
            tc.tile_pool(name="ps_o2", bufs=2, space="PSUM") as ps_o2,
        ):
            # staging tiles; only the zero region [W:1536) per head ever
            # needs zeroing (the window is fully overwritten every tile).
            # stg0's memset goes first on POOL so the zero-prefill DMAs for
            # tiles 0/1 can run while their windows are still computing.
            ZR = N_MOTIFS - W
            stgs = [persist.tile([128, H, N_MOTIFS], F32, tag=f"stg{i}",
                                 name=f"stg{i}") for i in range(3)]
            if ZR > 0:
                nc.gpsimd.memset(stgs[0][:, :, W:], 0.0)
            else:
                nc.gpsimd.memset(stgs[0][:], 0.0)

            cx0 = persist.tile([128, CX0F], F32)
            cx1 = persist.tile([128, CX1F], F32)
            cw = persist.tile([128, cwf], F32)
            c16 = persist.tile([128, c16f], BF16)
            rows = persist.tile([1, 1024], F32)
            # spread input loads across DMA paths so consumers wait only
            # on their own semaphore (SP-HWDGE / ACT-HWDGE / SWDGE), with
            # tile-0-critical data first
            nc.sync.dma_start(cw[:], cw_d[:])
            nc.sync.dma_start(cx0[:], cx0_d[:])
            nc.sync.dma_start(cx1[:], cx1_d[:])
            nc.scalar.dma_start(rows[:], rows_d[:])
            nc.gpsimd.dma_start(c16[:], c16_d[:])

            ident = cw[:, off_id:off_id + 128]
            x0 = cx0[:].rearrange("p (t d) -> p t d", t=4)
            x1 = cx1[:].rearrange("p (t d) -> p t d", t=T - 4)

            def x_tile(t):
                return x0[:, t, :] if t < 4 else x1[:, t - 4, :]
            mw = cw[:, off_mw:off_mw + NB * D].rearrange(
                "p (w d) -> p w d", w=NB)
            w_f32 = {
                "q": cw[:, off_wq:off_wq + 2 * D],
                "k": cw[:, off_wk:off_wk + 2 * D],
                "v": cw[:, off_wv:off_wv + 2 * D],
                "o": cw[:, off_wo:off_wo + 2 * D],
            }
            bq_col = cw[:, off_bq:off_bq + 2]
            bk_col = cw[:, off_bk:off_bk + 2]
            a_oh = c16[:, 0:NP]
            b_oh = c16[:, NP:NP + W]

            bv_row = rows[:, 0:D]
            bo_row = rows[:, D:2 * D]
            ones_row = rows[:, 2 * D:2 * D + 512]

            # ---- weights rounded to fp32r (one-time DVE copies)
            w_r = {}
            for key in ("q", "k", "v", "o"):
                wr = persist.tile([128, 2, D], F32R, name=f"w{key}r",
                                  tag=f"w{key}r")
                nc.scalar.activation(
                    wr[:], w_f32[key].rearrange("p (o d) -> p o d", o=2),
                    AF.Identity, bias=0.0)
                w_r[key] = wr

            # ---- prep (ordered so tile 0 unblocks earliest):
            # motif transposes -> kT/v, then per-512-chunk xT + qT
            mwT = persist.tile([128, 2, W], F32R)     # motif_win^T (rounded)
            for w in range(NB):
                for o in range(2):
                    trp = ps_tr.tile([128, 128], F32, tag="tr")
                    nc.tensor.transpose(
                        trp[:], mw[:, w, o * 128:(o + 1) * 128], ident)
                    nc.scalar.activation(
                        mwT[:, o, w * 128:(w + 1) * 128], trp[:],
                        AF.Identity, bias=0.0)

            # kT[dk, m] over the window
            kT = persist.tile([128, 2, W], F32R)
            for pt in range(2):
                ps_full = ps_sc.tile([128, 512], F32, tag="sc", name="kps")
                ps = ps_full[:, :W]
                nc.tensor.matmul(ps[:], w_r["k"][:, 0, pt * 128:(pt + 1) * 128],
                                 mwT[:, 0, :], start=True, stop=False)
                nc.tensor.matmul(ps[:], w_r["k"][:, 1, pt * 128:(pt + 1) * 128],
                                 mwT[:, 1, :], start=False, stop=True)
                nc.scalar.activation(kT[:, pt, :], ps[:], AF.Identity,
                                     bias=bk_col[:, pt:pt + 1])

            # ---- const bias tiles for ACT (only 0/1 are pre-registered)
            cm_neg = persist.tile([128, 1], F32)
            nc.vector.memset(cm_neg[:], -MASK_C)
            c_eps = persist.tile([128, 1], F32)
            nc.vector.memset(c_eps[:], LN_EPS)

            # remaining staging memsets + zero-region prefill for the
            # first tiles (keeps the DMA engines busy during prep)
            for i in (1, 2):
                if ZR > 0:
                    nc.gpsimd.memset(stgs[i][:, :, W:], 0.0)
                else:
                    nc.gpsimd.memset(stgs[i][:], 0.0)
            PREFILL = (0, 1) if ZR > 0 else ()
            for t in PREFILL:
                nc.sync.dma_start(
                    attn_d[t * 128:(t + 1) * 128, :, W:],
                    stgs[0][:, :, W:])

            res_all = persist.tile([128, T, D], F32)
            mv_all = persist.tile([128, T, 2], F32)

            xT = persist.tile([128, 2, NP], F32R)     # atom_x^T (rounded)
            qT = persist.tile([128, 2, NP], F32R)
            v_win = persist.tile([128, NB, D], F32)

            def emit_tile(t):
                stg = stgs[t % 3]
                n_sl = slice(t * 128, (t + 1) * 128)
                for h in range(H):
                    pt, po = h // 2, (h % 2) * 64
                    ps = ps_sc.tile([128, W], F32, tag="sc", name="scps")
                    nc.tensor.matmul(
                        ps[:, :W],
                        qT[po:po + 64, pt, n_sl],
                        kT[po:po + 64, pt, :], start=True, stop=False)
                    nc.tensor.matmul(ps[:, :W], a_oh[:, n_sl], b_oh[:],
                                     start=False, stop=True)
                    rs = work.tile([128, 1], F32, tag="rs", name="rs")
                    nc.scalar.activation(stg[:, h, 0:W], ps[:, :W], AF.Exp,
                                         bias=cm_neg[:], accum_out=rs[:])
                    iv = work.tile([128, 1], F32, tag="iv", name="iv")
                    nc.vector.reciprocal(iv[:], rs[:])
                    nc.vector.tensor_scalar_mul(stg[:, h, 0:W],
                                                stg[:, h, 0:W], iv[:])

                # attn^T blocks + attn @ v  (normalized attn already in stg)
                aoT = ps_ao.tile([128, 2, 128], F32, tag="ao", name="aoT")
                for h in range(H):
                    for w in range(NB):
                        trp = ps_tr.tile([128, 128], F32, tag="tr", name="trp")
                        nc.tensor.transpose(
                            trp[:], stg[:, h, w * 128:(w + 1) * 128], ident)
                        expT = work.tile([128, 128], F32, tag="expT", name="expT")
                        nc.vector.tensor_copy(expT[:], trp[:])
                        nc.tensor.matmul(
                            aoT[(h % 2) * 64:(h % 2) * 64 + 64, h // 2, :],
                            v_win[:, w, h * HD:(h + 1) * HD],
                            expT[:],
                            start=(w == 0), stop=(w == NB - 1),
                            tile_position=(0, (h % 2) * 64))
                aoT_sb = work.tile([128, 2, 128], F32R, tag="aoT_sb", name="aoT_sb")
                nc.vector.tensor_copy(aoT_sb[:], aoT[:])

                # out2 = attn_out @ Wo^T + bo
                o2 = ps_o2.tile([128, D], F32, tag="o2", name="o2")
                nc.tensor.matmul(o2[:], aoT_sb[:, 0, :], w_r["o"][:, 0, :],
                                 start=True, stop=False)
                nc.tensor.matmul(o2[:], aoT_sb[:, 1, :], w_r["o"][:, 1, :],
                                 start=False, stop=False)
                nc.tensor.matmul(o2[:], ones_row[:, 0:128], bo_row[:],
                                 start=False, stop=True)

                # residual + LN stats (sqrt/normalize deferred to tail
                # so ACT never swaps tables Exp<->Sqrt mid-stream)
                nc.vector.tensor_tensor(res_all[:, t, :], o2[:],
                                        x_tile(t), ALU.add)
                st6 = work.tile([128, 6], F32, tag="st6", name="st6")
                nc.vector.bn_stats(st6[:], res_all[:, t, :])
                nc.vector.bn_aggr(mv_all[:, t, :], st6[:])
                if t in PREFILL:
                    nc.sync.dma_start(attn_d[n_sl, :, 0:W], stg[:, :, 0:W])
                else:
                    nc.sync.dma_start(attn_d[n_sl, :, :], stg[:])


            # xT + qT per 512-chunk, interleaved with that chunk's 4 tiles
            # so tile 0's attn DMA starts as early as possible
            for ch in range(NP // 512):
                for t in range(4 * ch, 4 * ch + 4):
                    for o in range(2):
                        trp = ps_tr.tile([128, 128], F32, tag="tr",
                                         name="trpx")
                        nc.tensor.transpose(
                            trp[:], x_tile(t)[:, o * 128:(o + 1) * 128],
                            ident)
                        nc.vector.tensor_copy(
                            xT[:, o, t * 128:(t + 1) * 128], trp[:])
                for pt in range(2):
                    ps = ps_sc.tile([128, 512], F32, tag="sc", name="qps")
                    nc.tensor.matmul(
                        ps[:], w_r["q"][:, 0, pt * 128:(pt + 1) * 128],
                        xT[:, 0, ch * 512:(ch + 1) * 512],
                        start=True, stop=False)
                    nc.tensor.matmul(
                        ps[:], w_r["q"][:, 1, pt * 128:(pt + 1) * 128],
                        xT[:, 1, ch * 512:(ch + 1) * 512],
                        start=False, stop=True)
                    nc.scalar.activation(
                        qT[:, pt, ch * 512:(ch + 1) * 512], ps[:],
                        AF.Identity, bias=bq_col[:, pt:pt + 1])
                if ch == 0:
                    # v natural [m, dv] over the window (bias via K=1 ones-matmul)
                    for w in range(NB):
                        ps = ps_o2.tile([128, D], F32, tag="o2")
                        nc.tensor.matmul(ps[:], mwT[:, 0, w * 128:(w + 1) * 128],
                                                 w_r["v"][:, 0, :], start=True, stop=False)
                        nc.tensor.matmul(ps[:], mwT[:, 1, w * 128:(w + 1) * 128],
                                                 w_r["v"][:, 1, :], start=False, stop=False)
                        nc.tensor.matmul(ps[:], ones_row[:, 0:128], bv_row[:],
                                                 start=False, stop=True)
                        nc.vector.tensor_copy(v_win[:, w, :], ps[:])

                for t in range(4 * ch, 4 * ch + 4):
                    emit_tile(t)

            # ---- LN tail: batched sqrt/reciprocal, then one out DMA
            sd_all = work.tile([128, T], F32, tag="sd_all")
            nc.scalar.activation(sd_all[:], mv_all[:, :, 1], AF.Sqrt,
                                 bias=c_eps[:])
            rstd_all = work.tile([128, T], F32, tag="rstd_all")
            nc.vector.reciprocal(rstd_all[:], sd_all[:])
            murs_all = work.tile([128, T], F32, tag="murs_all")
            nc.vector.tensor_tensor(murs_all[:], mv_all[:, :, 0],
                                    rstd_all[:], ALU.mult)
            out_all = persist.tile([128, T, D], F32)
            for t in range(T):
                nc.vector.tensor_scalar(out_all[:, t, :], res_all[:, t, :],
                                        rstd_all[:, t:t + 1],
                                        murs_all[:, t:t + 1],
                                        ALU.mult, ALU.subtract)
            nc.sync.dma_start(
                out_d.rearrange("(t p) d -> p t d", p=128), out_all[:])

    nc.compile()
    return nc


def prepare(atom_x, motif_x, atom_batch, motif_batch,
            Wq, bq, Wk, bk, Wv, bv, Wo, bo, ln_g, ln_b):
    """Host prep: build program + per-core input maps.  Returns
    (nc, in_maps, meta) where meta carries what unshard() needs."""
    atom_x = np.ascontiguousarray(np.asarray(atom_x, dtype=np.float32))
    motif_x = np.ascontiguousarray(np.asarray(motif_x, dtype=np.float32))
    ab = np.asarray(atom_batch).astype(np.int64)
    mb = np.asarray(motif_batch).astype(np.int64)
    Wq = np.asarray(Wq, dtype=np.float32)
    Wk = np.asarray(Wk, dtype=np.float32)
    Wv = np.asarray(Wv, dtype=np.float32)
    Wo = np.asarray(Wo, dtype=np.float32)
    bq = np.asarray(bq, dtype=np.float32)
    bk = np.asarray(bk, dtype=np.float32)
    bv = np.asarray(bv, dtype=np.float32)
    bo = np.asarray(bo, dtype=np.float32)
    ln_g = np.asarray(ln_g, dtype=np.float32)
    ln_b = np.asarray(ln_b, dtype=np.float32)

    sorted_ok = (np.all(np.diff(ab) >= 0) and np.all(np.diff(mb) >= 0))
    G = 128
    ms = np.searchsorted(mb, np.arange(G), "left")
    me = np.searchsorted(mb, np.arange(G), "right")

    # per-core motif windows
    los, spans = [], []
    for c in range(N_CORES):
        if sorted_ok:
            g0 = int(ab[c * NP])
            g1 = int(ab[(c + 1) * NP - 1])
            lo = int(ms[g0])
            hi = int(me[g1])
            if hi <= lo:  # no motifs at all for these graphs
                lo, hi = 0, 1
        else:
            lo, hi = 0, N_MOTIFS
        los.append(lo)
        spans.append(hi - lo)
    W = int(-(-max(spans) // 128) * 128)
    W = min(max(W, 128), N_MOTIFS)

    nc = _build_program(W)

    NB = W // 128
    off_id = 0
    off_mw = off_id + 128
    off_wq = off_mw + NB * D
    off_wk = off_wq + 2 * D
    off_wv = off_wk + 2 * D
    off_wo = off_wv + 2 * D
    off_bq = off_wo + 2 * D
    off_bk = off_bq + 2
    cwf = off_bk + 2
    c16f = NP + W

    def t_layout(M):  # [256, 256] -> [128, 2, 256] with (o p) split
        return np.ascontiguousarray(
            M.T.reshape(2, 128, D).transpose(1, 0, 2))

    cw_h = np.zeros((128, cwf), np.float32)
    cw_h[:, off_id:off_id + 128] = np.eye(128, dtype=np.float32)
    cw_h[:, off_wq:off_wq + 2 * D] = t_layout(Wq / 8.0).reshape(128, 2 * D)
    cw_h[:, off_wk:off_wk + 2 * D] = t_layout(Wk).reshape(128, 2 * D)
    cw_h[:, off_wv:off_wv + 2 * D] = t_layout(Wv).reshape(128, 2 * D)
    cw_h[:, off_wo:off_wo + 2 * D] = t_layout(Wo).reshape(128, 2 * D)
    cw_h[:, off_bq:off_bq + 2] = (bq / 8.0).reshape(2, 128).T
    cw_h[:, off_bk:off_bk + 2] = bk.reshape(2, 128).T

    rows_h = np.zeros((1, 1024), np.float32)
    rows_h[0, 0:D] = bv
    rows_h[0, D:2 * D] = bo
    rows_h[0, 2 * D:2 * D + 512] = 1.0

    in_maps = []
    for c in range(N_CORES):
        lo = los[c]
        xs = atom_x[c * NP:(c + 1) * NP]  # [1536, 256]
        xs_p = xs.reshape(T, 128, D).transpose(1, 0, 2)  # [128, T, D]
        cx0 = np.ascontiguousarray(xs_p[:, :4].reshape(128, CX0F))
        cx1 = np.ascontiguousarray(xs_p[:, 4:].reshape(128, CX1F))

        cw = cw_h.copy()
        mwin = np.zeros((W, D), np.float32)
        valid = min(W, N_MOTIFS - lo)
        mwin[:valid] = motif_x[lo:lo + valid]
        cw[:, off_mw:off_mw + NB * D] = (
            mwin.reshape(NB, 128, D).transpose(1, 0, 2).reshape(128, NB * D))

        c16 = np.zeros((128, c16f), ml_dtypes.bfloat16)
        abc = ab[c * NP:(c + 1) * NP]
        c16[:, 0:NP] = ((abc[None, :] == np.arange(G)[:, None]) *
                        MASK_C).astype(ml_dtypes.bfloat16)
        mbw = np.full(W, -1, np.int64)
        mbw[:valid] = mb[lo:lo + valid]
        c16[:, NP:NP + W] = (
            mbw[None, :] == np.arange(G)[:, None]).astype(ml_dtypes.bfloat16)
        in_maps.append({"combo_x0": cx0, "combo_x1": cx1, "combo_w": cw,
                        "combo16": c16, "rows": rows_h})

    meta = {
        "los": los, "ms": ms, "me": me, "sorted_ok": sorted_ok,
        "ab": ab, "atom_x": atom_x, "motif_x": motif_x,
        "Wq": Wq, "bq": bq, "Wv": Wv, "bv": bv, "Wo": Wo, "bo": bo,
        "ln_g": ln_g, "ln_b": ln_b,
    }
    return nc, in_maps, meta


def unshard(results, meta):
    """Assemble per-core device results into the full (out, attn)."""
    los = meta["los"]
    out = np.empty((N_ATOMS, D), np.float32)
    attn = np.empty((N_ATOMS, H, N_MOTIFS), np.float32)
    for c in range(N_CORES):
        r = results[c]
        out[c * NP:(c + 1) * NP] = r["out"]
        a = r["attn_out"]  # [NP, H, 1536], window at cols [0, W)
        lo = los[c]
        dst = attn[c * NP:(c + 1) * NP]
        if lo == 0:
            dst[:] = a
        else:
            dst[:, :, lo:] = a[:, :, :N_MOTIFS - lo]
            dst[:, :, :lo] = a[:, :, N_MOTIFS - lo:]

    ln_g, ln_b = meta["ln_g"], meta["ln_b"]
    # device computes LN with g=1, b=0; apply g/b here if nontrivial
    if not (np.allclose(ln_g, 1.0) and np.allclose(ln_b, 0.0)):
        out = out * ln_g + ln_b

    # fallback for rows whose graph has no motifs (reference: uniform attn)
    if meta["sorted_ok"]:
        ms, me, ab = meta["ms"], meta["me"], meta["ab"]
        empty = np.where(me == ms)[0]
        if len(empty):
            bad = np.isin(ab, empty)
            if bad.any():
                idx = np.where(bad)[0]
                atom_x, motif_x = meta["atom_x"], meta["motif_x"]
                Wv, bv, Wo, bo = (meta[k] for k in ["Wv", "bv", "Wo", "bo"])
                attn[idx] = 1.0 / N_MOTIFS
                v_all = motif_x @ Wv.T + bv
                o = np.einsum("nhm,mhd->nhd",
                              attn[idx],
                              v_all.reshape(N_MOTIFS, H, HD)).reshape(
                                  len(idx), D)
                y = o @ Wo.T + bo + atom_x[idx]
                mu = y.mean(-1, keepdims=True)
                var = ((y - mu) ** 2).mean(-1, keepdims=True)
                out[idx] = (y - mu) / np.sqrt(var + LN_EPS) * \
                    meta["ln_g"] + meta["ln_b"]

    return out, attn


def _host_reference(atom_x, motif_x, atom_batch, motif_batch,
                    Wq, bq, Wk, bk, Wv, bv, Wo, bo, ln_g, ln_b):
    """Exact numpy fallback for pathological inputs (unsorted batches or
    window > 512) that the device path does not handle."""
    n = atom_x.shape[0]
    m = motif_x.shape[0]
    q = (atom_x @ Wq.T + bq).reshape(n, H, HD)
    k = (motif_x @ Wk.T + bk).reshape(m, H, HD)
    v = (motif_x @ Wv.T + bv).reshape(m, H, HD)
    scores = np.einsum("nhd,mhd->nhm", q, k) / np.sqrt(np.float32(HD))
    mask = atom_batch[:, None] == motif_batch[None, :]
    scores = np.where(mask[:, None, :], scores, np.float32(-1e9))
    scores -= scores.max(-1, keepdims=True)
    e = np.exp(scores)
    attn = (e / e.sum(-1, keepdims=True)).astype(np.float32)
    out = np.einsum("nhm,mhd->nhd", attn, v).reshape(n, H * HD)
    out = out @ Wo.T + bo + atom_x
    mu = out.mean(-1, keepdims=True)
    var = ((out - mu) ** 2).mean(-1, keepdims=True)
    out = ((out - mu) / np.sqrt(var + LN_EPS) * ln_g + ln_b).astype(
        np.float32)
    return out, attn


def kernel(**inputs):
    ab = np.asarray(inputs["atom_batch"]).astype(np.int64)
    mb = np.asarray(inputs["motif_batch"]).astype(np.int64)
    if np.all(np.diff(ab) >= 0) and np.all(np.diff(mb) >= 0):
        ms = np.searchsorted(mb, np.arange(128), "left")
        me = np.searchsorted(mb, np.arange(128), "right")
        span = max(
            int(me[ab[(c + 1) * NP - 1]]) - int(ms[ab[c * NP]])
            for c in range(N_CORES))
        device_ok = span <= 512
    else:
        device_ok = False
    if not device_ok:
        a = {k2: np.asarray(v2, dtype=np.float32) for k2, v2 in
             inputs.items() if k2 not in ("atom_batch", "motif_batch")}
        return _host_reference(atom_batch=ab, motif_batch=mb, **a)

    nc, in_maps, meta = prepare(**inputs)
    trace = bool(int(os.environ.get("BASS_KERNEL_TRACE", "0")))
    res = run_bass_kernel_spmd(nc, in_maps, core_ids=list(range(N_CORES)),
                               trace=trace)
    if trace and res.exec_time_ns is not None:
        print(f"HW exec time: {res.exec_time_ns} ns")
    return unshard(res.results, meta)


# revision 30
# speedup vs baseline: 1.0259x; 1.0259x over previous
"""AtomMotifAttention Trainium2 kernel.

Strategy (8 NeuronCores, SPMD single program):
  - Shard atoms: core c handles rows [c*1536, (c+1)*1536).
  - Both batch vectors are sorted, so the attention is block-diagonal: the
    atoms of one core only attend to a narrow contiguous window of motifs
    (max span ~235 of 1536 here).  Each core gets a host-sliced window of
    motif_x (W columns, W = max span over cores rounded to 128), computes
    scores/softmax only on that window, and assembles each 128-row tile of
    attn in a [128, 4*1536] staging tile whose columns [h*1536, h*1536+W)
    hold the window and the rest are zeros.  The full 302 MB attn tensor
    is written by the device; the host only rolls each core's block by its
    window start during unsharding (pure memcpy of device-produced bytes).
  - The batch mask is folded into the scores matmul as a one-hot bf16
    accumulation: psum = q.k/8 + 1000*eq, exp(psum - 1000) underflows to
    exactly 0 for masked entries (same as the reference, where
    exp(-1e9) == 0) and subtract-max is unnecessary because every graph
    has >=1 motif and scores are O(10).
  - attn @ v needs attn^T: PE-transpose the normalized window tiles.
  - Hot matmuls run in float32r (4x faster than float32 on TRN2 for
    moving dim >= 256; measured exact for these magnitudes).  float32r
    operands must be produced "rounded": the psum->sbuf copybacks (DVE
    tensor_copy / ACT Identity) write float32r tiles directly.
  - Q/K biases ride the ACT-Identity copyback (per-partition bias), V/O
    biases are K=1 ones-matmuls accumulated into psum.
  - Residual + LayerNorm fused on-chip; ln_g/ln_b applied on host only if
    nontrivial (they are 1/0 here; (y-mu)*rstd*g+b is exact either way).

Compile-time constraints worked around:
  - Each PE instruction supports exactly ONE fresh semaphore wait, so PE
    inputs are packed into few big DMAs and every PE-feeding producer
    keeps a PE instruction's fresh deps on a single semaphore.
  - bacc.Bacc + nc.compile() is required (raw bass.Bass BIR fails walrus
    codegen on multi-wait instructions).
"""

import os
import sys

import numpy as np

sys.path.insert(0, "/opt/trn_rl_repo")

import ml_dtypes

import concourse.bacc as bacc
import concourse.mybir as mybir
import concourse.tile as tile
from concourse.bass_utils import run_bass_kernel_spmd

F32 = mybir.dt.float32
F32R = mybir.dt.float32r
BF16 = mybir.dt.bfloat16
AF = mybir.ActivationFunctionType
ALU = mybir.AluOpType

N_CORES = 8
N_ATOMS = 12288
N_MOTIFS = 1536
D = 256
H = 4
HD = 64
NP = N_ATOMS // N_CORES      # rows per core = 1536
T = NP // 128                # 12 tiles of 128 rows per core
MASK_C = 1000.0
LN_EPS = 1e-5

# combo_x split: cx0 = tiles 0-3 (early), cx1 = tiles 4-11
CX0F = 4 * D
CX1F = (T - 4) * D


def _build_program(W):
    """Build the SPMD bass program for window width W (multiple of 128)."""
    NB = W // 128  # motif window blocks

    # combo_w layout (f32): identity, motif window, 4 weights, bias columns
    off_id = 0
    off_mw = off_id + 128
    off_wq = off_mw + NB * D
    off_wk = off_wq + 2 * D
    off_wv = off_wk + 2 * D
    off_wo = off_wv + 2 * D
    off_bq = off_wo + 2 * D         # [128, 2] columns of bq/8
    off_bk = off_bq + 2             # [128, 2] columns of bk
    cwf = off_bk + 2
    # combo16 layout (bf16): A1000 [128, NP], B [128, W]
    c16f = NP + W

    nc = bacc.Bacc(None, target_bir_lowering=False)

    cx0_d = nc.dram_tensor("combo_x0", [128, CX0F], F32,
                           kind="ExternalInput")
    cx1_d = nc.dram_tensor("combo_x1", [128, CX1F], F32,
                           kind="ExternalInput")
    cw_d = nc.dram_tensor("combo_w", [128, cwf], F32, kind="ExternalInput")
    c16_d = nc.dram_tensor("combo16", [128, c16f], BF16, kind="ExternalInput")
    # rows: [bv(256), bo(256), ones(512)] on partition 0
    rows_d = nc.dram_tensor("rows", [1, 1024], F32, kind="ExternalInput")
    attn_d = nc.dram_tensor("attn_out", [NP, H, N_MOTIFS], F32,
                            kind="ExternalOutput")
    out_d = nc.dram_tensor("out", [NP, D], F32, kind="ExternalOutput")

    with tile.TileContext(nc) as tc:
        with (
            tc.tile_pool(name="persist", bufs=1) as persist,
            tc.tile_pool(name="work", bufs=3) as work,
            tc.tile_pool(name="ps_sc", bufs=2, space="PSUM") as ps_sc,
            tc.tile_pool(name="ps_tr", bufs=2, space="PSUM") as ps_tr,
            tc.tile_pool(name="ps_ao", bufs=2, space="PSUM") as ps_ao,
            tc.tile_pool(name="ps_o2", bufs=2, space="PSUM") as ps_o2,
        ):
            # staging tiles; only the zero region [W:1536) per head ever
            # needs zeroing (the window is fully overwritten every tile).
            # stg0's memset goes first on POOL so the zero-prefill DMAs for
            # tiles 0/1 can run while their windows are still computing.
            ZR = N_MOTIFS - W
            stgs = [persist.tile([128, H, N_MOTIFS], F32, tag=f"stg{i}",
                                 name=f"stg{i}") for i in range(3)]
            if ZR > 0:
                nc.gpsimd.memset(stgs[0][:, :, W:], 0.0)
            else:
                nc.gpsimd.memset(stgs[0][:], 0.0)

            cx0 = persist.tile([128, CX0F], F32)
            cx1 = persist.tile([128, CX1F], F32)
            cw = persist.tile([128, cwf], F32)
            c16 = persist.tile([128, c16f], BF16)
            rows = persist.tile([1, 1024], F32)
            # spread input loads across DMA paths so consumers wait only
            # on their own semaphore (SP-HWDGE / ACT-HWDGE / SWDGE), with
            # tile-0-critical data first
            nc.sync.dma_start(cw[:], cw_d[:])
            nc.sync.dma_start(cx0[:], cx0_d[:])
            nc.sync.dma_start(cx1[:], cx1_d[:])
            nc.scalar.dma_start(rows[:], rows_d[:])
            nc.gpsimd.dma_start(c16[:], c16_d[:])

            ident = cw[:, off_id:off_id + 128]
            x0 = cx0[:].rearrange("p (t d) -> p t d", t=4)
            x1 = cx1[:].rearrange("p (t d) -> p t d", t=T - 4)

            def x_tile(t):
                return x0[:, t, :] if t < 4 else x1[:, t - 4, :]
            mw = cw[:, off_mw:off_mw + NB * D].rearrange(
                "p (w d) -> p w d", w=NB)
            w_f32 = {
                "q": cw[:, off_wq:off_wq + 2 * D],
                "k": cw[:, off_wk:off_wk + 2 * D],
                "v": cw[:, off_wv:off_wv + 2 * D],
                "o": cw[:, off_wo:off_wo + 2 * D],
            }
            bq_col = cw[:, off_bq:off_bq + 2]
            bk_col = cw[:, off_bk:off_bk + 2]
            a_oh = c16[:, 0:NP]
            b_oh = c16[:, NP:NP + W]

            bv_row = rows[:, 0:D]
            bo_row = rows[:, D:2 * D]
            ones_row = rows[:, 2 * D:2 * D + 512]

            # ---- weights rounded to fp32r (one-time DVE copies)
            w_r = {}
            for key in ("q", "k", "v", "o"):
                wr = persist.tile([128, 2, D], F32R, name=f"w{key}r",
                                  tag=f"w{key}r")
                nc.scalar.activation(
                    wr[:], w_f32[key].rearrange("p (o d) -> p o d", o=2),
                    AF.Identity, bias=0.0)
                w_r[key] = wr

            # ---- prep (ordered so tile 0 unblocks earliest):
            # motif transposes -> kT/v, then per-512-chunk xT + qT
            mwT = persist.tile([128, 2, W], F32R)     # motif_win^T (rounded)
            for w in range(NB):
                for o in range(2):
                    trp = ps_tr.tile([128, 128], F32, tag="tr")
                    nc.tensor.transpose(
                        trp[:], mw[:, w, o * 128:(o + 1) * 128], ident)
                    nc.scalar.activation(
                        mwT[:, o, w * 128:(w + 1) * 128], trp[:],
                        AF.Identity, bias=0.0)

            # kT[dk, m] over the window
            kT = persist.tile([128, 2, W], F32R)
            for pt in range(2):
                ps_full = ps_sc.tile([128, 512], F32, tag="sc", name="kps")
                ps = ps_full[:, :W]
                nc.tensor.matmul(ps[:], w_r["k"][:, 0, pt * 128:(pt + 1) * 128],
                                 mwT[:, 0, :], start=True, stop=False)
                nc.tensor.matmul(ps[:], w_r["k"][:, 1, pt * 128:(pt + 1) * 128],
                                 mwT[:, 1, :], start=False, stop=True)
                nc.scalar.activation(kT[:, pt, :], ps[:], AF.Identity,
                                     bias=bk_col[:, pt:pt + 1])

            # ---- const bias tiles for ACT (only 0/1 are pre-registered)
            cm_neg = persist.tile([128, 1], F32)
            nc.vector.memset(cm_neg[:], -MASK_C)
            c_eps = persist.tile([128, 1], F32)
            nc.vector.memset(c_eps[:], LN_EPS)

            # remaining staging memsets + zero-region prefill for the
            # first tiles (keeps the DMA engines busy during prep)
            for i in (1, 2):
                if ZR > 0:
                    nc.gpsimd.memset(stgs[i][:, :, W:], 0.0)
                else:
                    nc.gpsimd.memset(stgs[i][:], 0.0)
            PREFILL = (0, 1) if ZR > 0 else ()
            for t in PREFILL:
                nc.sync.dma_start(
                    attn_d[t * 128:(t + 1) * 128, :, W:],
                    stgs[0][:, :, W:])

            res_all = persist.tile([128, T, D], F32)
            mv_all = persist.tile([128, T, 2], F32)

            xT = persist.tile([128, 2, NP], F32R)     # atom_x^T (rounded)
            qT = persist.tile([128, 2, NP], F32R)
            v_win = persist.tile([128, NB, D], F32)

            def emit_tile(t):
                stg = stgs[t % 3]
                n_sl = slice(t * 128, (t + 1) * 128)
                for h in range(H):
                    pt, po = h // 2, (h % 2) * 64
                    ps = ps_sc.tile([128, W], F32, tag="sc", name="scps")
                    nc.tensor.matmul(
                        ps[:, :W],
                        qT[po:po + 64, pt, n_sl],
                        kT[po:po + 64, pt, :], start=True, stop=False)
                    nc.tensor.matmul(ps[:, :W], a_oh[:, n_sl], b_oh[:],
                                     start=False, stop=True)
                    rs = work.tile([128, 1], F32, tag="rs", name="rs")
                    nc.scalar.activation(stg[:, h, 0:W], ps[:, :W], AF.Exp,
                                         bias=cm_neg[:], accum_out=rs[:])
                    iv = work.tile([128, 1], F32, tag="iv", name="iv")
                    nc.vector.reciprocal(iv[:], rs[:])
                    nc.vector.tensor_scalar_mul(stg[:, h, 0:W],
                                                stg[:, h, 0:W], iv[:])

                # attn^T blocks + attn @ v  (normalized attn already in stg)
                aoT = ps_ao.tile([128, 2, 128], F32, tag="ao", name="aoT")
                for h in range(H):
                    for w in range(NB):
                        trp = ps_tr.tile([128, 128], F32, tag="tr", name="trp")
                        nc.tensor.transpose(
                            trp[:], stg[:, h, w * 128:(w + 1) * 128], ident)
                        expT = work.tile([128, 128], F32, tag="expT", name="expT")
                        nc.vector.tensor_copy(expT[:], trp[:])
                        nc.tensor.matmul(
                            aoT[(h % 2) * 64:(h % 2) * 64 + 64, h // 2, :],
                            v_win[:, w, h * HD:(h + 1) * HD],
                            expT[:],
                            start=(w == 0), stop=(w == NB - 1),
                            tile_position=(0, (h % 2) * 64))
                aoT_sb = work.tile([128, 2, 128], F32R, tag="aoT_sb", name="aoT_sb")
                nc.vector.tensor_copy(aoT_sb[:], aoT[:])

                # out2 = attn_out @ Wo^T + bo
                o2 = ps_o2.tile([128, D], F32, tag="o2", name="o2")
                nc.tensor.matmul(o2[:], aoT_sb[:, 0, :], w_r["o"][:, 0, :],
                                 start=True, stop=False)
                nc.tensor.matmul(o2[:], aoT_sb[:, 1, :], w_r["o"][:, 1, :],
                                 start=False, stop=False)
                nc.tensor.matmul(o2[:], ones_row[:, 0:128], bo_row[:],
                                 start=False, stop=True)

                # residual + LN stats (sqrt/normalize deferred to tail
                # so ACT never swaps tables Exp<->Sqrt mid-stream)
                nc.vector.tensor_tensor(res_all[:, t, :], o2[:],
                                        x_tile(t), ALU.add)
                st6 = work.tile([128, 6], F32, tag="st6", name="st6")
                nc.vector.bn_stats(st6[:], res_all[:, t, :])
                nc.vector.bn_aggr(mv_all[:, t, :], st6[:])
                if t in PREFILL:
                    nc.sync.dma_start(attn_d[n_sl, :, 0:W], stg[:, :, 0:W])
                elif t in (2, 4) and ZR > 0:
                    nc.sync.dma_start(attn_d[n_sl, 0:2, 0:W],
                                      stg[:, 0:2, 0:W])
                    nc.sync.dma_start(attn_d[n_sl, 2:4, :], stg[:, 2:4, :])
                else:
                    nc.sync.dma_start(attn_d[n_sl, :, :], stg[:])


            # xT + qT per 512-chunk, interleaved with that chunk's 4 tiles
            # so tile 0's attn DMA starts as early as possible
            for ch in range(NP // 512):
                for t in range(4 * ch, 4 * ch + 4):
                    for o in range(2):
                        trp = ps_tr.tile([128, 128], F32, tag="tr",
                                         name="trpx")
                        nc.tensor.transpose(
                            trp[:], x_tile(t)[:, o * 128:(o + 1) * 128],
                            ident)
                        nc.vector.tensor_copy(
                            xT[:, o, t * 128:(t + 1) * 128], trp[:])
                for pt in range(2):
                    ps = ps_sc.tile([128, 512], F32, tag="sc", name="qps")
                    nc.tensor.matmul(
                        ps[:], w_r["q"][:, 0, pt * 128:(pt + 1) * 128],
                        xT[:, 0, ch * 512:(ch + 1) * 512],
                        start=True, stop=False)
                    nc.tensor.matmul(
                        ps[:], w_r["q"][:, 1, pt * 128:(pt + 1) * 128],
                        xT[:, 1, ch * 512:(ch + 1) * 512],
                        start=False, stop=True)
                    nc.scalar.activation(
                        qT[:, pt, ch * 512:(ch + 1) * 512], ps[:],
                        AF.Identity, bias=bq_col[:, pt:pt + 1])
                if ch == 0:
                    # v natural [m, dv] over the window (bias via K=1 ones-matmul)
                    for w in range(NB):
                        ps = ps_o2.tile([128, D], F32, tag="o2")
                        nc.tensor.matmul(ps[:], mwT[:, 0, w * 128:(w + 1) * 128],
                                                 w_r["v"][:, 0, :], start=True, stop=False)
                        nc.tensor.matmul(ps[:], mwT[:, 1, w * 128:(w + 1) * 128],
                                                 w_r["v"][:, 1, :], start=False, stop=False)
                        nc.tensor.matmul(ps[:], ones_row[:, 0:128], bv_row[:],
                                                 start=False, stop=True)
                        nc.vector.tensor_copy(v_win[:, w, :], ps[:])

                for t in range(4 * ch, 4 * ch + 4):
                    emit_tile(t)
                    if ch == 0 and t == 0 and ZR > 0:
                        nc.sync.dma_start(
                            attn_d[2 * 128:3 * 128, 0:2, W:],
                            stgs[0][:, 0:2, W:])
                    if ch == 0 and t == 2 and ZR > 0:
                        nc.sync.dma_start(
                            attn_d[4 * 128:5 * 128, 0:2, W:],
                            stgs[0][:, 0:2, W:])

            # ---- LN tail: batched sqrt/reciprocal, then one out DMA
            sd_all = work.tile([128, T], F32, tag="sd_all")
            nc.scalar.activation(sd_all[:], mv_all[:, :, 1], AF.Sqrt,
                                 bias=c_eps[:])
            rstd_all = work.tile([128, T], F32, tag="rstd_all")
            nc.vector.reciprocal(rstd_all[:], sd_all[:])
            murs_all = work.tile([128, T], F32, tag="murs_all")
            nc.vector.tensor_tensor(murs_all[:], mv_all[:, :, 0],
                                    rstd_all[:], ALU.mult)
            out_all = persist.tile([128, T, D], F32)
            for t in range(T):
                nc.vector.tensor_scalar(out_all[:, t, :], res_all[:, t, :],
                                        rstd_all[:, t:t + 1],
                                        murs_all[:, t:t + 1],
                                        ALU.mult, ALU.subtract)
            nc.sync.dma_start(
                out_d.rearrange("(t p) d -> p t d", p=128), out_all[:])

    nc.compile()
    return nc


def prepare(atom_x, motif_x, atom_batch, motif_batch,
            Wq, bq, Wk, bk, Wv, bv, Wo, bo, ln_g, ln_b):
    """Host prep: build program + per-core input maps.  Returns
    (nc, in_maps, meta) where meta carries what unshard() needs."""
    atom_x = np.ascontiguousarray(np.asarray(atom_x, dtype=np.float32))
    motif_x = np.ascontiguousarray(np.asarray(motif_x, dtype=np.float32))
    ab = np.asarray(atom_batch).astype(np.int64)
    mb = np.asarray(motif_batch).astype(np.int64)
    Wq = np.asarray(Wq, dtype=np.float32)
    Wk = np.asarray(Wk, dtype=np.float32)
    Wv = np.asarray(Wv, dtype=np.float32)
    Wo = np.asarray(Wo, dtype=np.float32)
    bq = np.asarray(bq, dtype=np.float32)
    bk = np.asarray(bk, dtype=np.float32)
    bv = np.asarray(bv, dtype=np.float32)
    bo = np.asarray(bo, dtype=np.float32)
    ln_g = np.asarray(ln_g, dtype=np.float32)
    ln_b = np.asarray(ln_b, dtype=np.float32)

    sorted_ok = (np.all(np.diff(ab) >= 0) and np.all(np.diff(mb) >= 0))
    G = 128
    ms = np.searchsorted(mb, np.arange(G), "left")
    me = np.searchsorted(mb, np.arange(G), "right")

    # per-core motif windows
    los, spans = [], []
    for c in range(N_CORES):
        if sorted_ok:
            g0 = int(ab[c * NP])
            g1 = int(ab[(c + 1) * NP - 1])
            lo = int(ms[g0])
            hi = int(me[g1])
            if hi <= lo:  # no motifs at all for these graphs
                lo, hi = 0, 1
        else:
            lo, hi = 0, N_MOTIFS
        los.append(lo)
        spans.append(hi - lo)
    W = int(-(-max(spans) // 128) * 128)
    W = min(max(W, 128), N_MOTIFS)

    nc = _build_program(W)

    NB = W // 128
    off_id = 0
    off_mw = off_id + 128
    off_wq = off_mw + NB * D
    off_wk = off_wq + 2 * D
    off_wv = off_wk + 2 * D
    off_wo = off_wv + 2 * D
    off_bq = off_wo + 2 * D
    off_bk = off_bq + 2
    cwf = off_bk + 2
    c16f = NP + W

    def t_layout(M):  # [256, 256] -> [128, 2, 256] with (o p) split
        return np.ascontiguousarray(
            M.T.reshape(2, 128, D).transpose(1, 0, 2))

    cw_h = np.zeros((128, cwf), np.float32)
    cw_h[:, off_id:off_id + 128] = np.eye(128, dtype=np.float32)
    cw_h[:, off_wq:off_wq + 2 * D] = t_layout(Wq / 8.0).reshape(128, 2 * D)
    cw_h[:, off_wk:off_wk + 2 * D] = t_layout(Wk).reshape(128, 2 * D)
    cw_h[:, off_wv:off_wv + 2 * D] = t_layout(Wv).reshape(128, 2 * D)
    cw_h[:, off_wo:off_wo + 2 * D] = t_layout(Wo).reshape(128, 2 * D)
    cw_h[:, off_bq:off_bq + 2] = (bq / 8.0).reshape(2, 128).T
    cw_h[:, off_bk:off_bk + 2] = bk.reshape(2, 128).T

    rows_h = np.zeros((1, 1024), np.float32)
    rows_h[0, 0:D] = bv
    rows_h[0, D:2 * D] = bo
    rows_h[0, 2 * D:2 * D + 512] = 1.0

    in_maps = []
    for c in range(N_CORES):
        lo = los[c]
        xs = atom_x[c * NP:(c + 1) * NP]  # [1536, 256]
        xs_p = xs.reshape(T, 128, D).transpose(1, 0, 2)  # [128, T, D]
        cx0 = np.ascontiguousarray(xs_p[:, :4].reshape(128, CX0F))
        cx1 = np.ascontiguousarray(xs_p[:, 4:].reshape(128, CX1F))

        cw = cw_h.copy()
        mwin = np.zeros((W, D), np.float32)
        valid = min(W, N_MOTIFS - lo)
        mwin[:valid] = motif_x[lo:lo + valid]
        cw[:, off_mw:off_mw + NB * D] = (
            mwin.reshape(NB, 128, D).transpose(1, 0, 2).reshape(128, NB * D))

        c16 = np.zeros((128, c16f), ml_dtypes.bfloat16)
        abc = ab[c * NP:(c + 1) * NP]
        c16[:, 0:NP] = ((abc[None, :] == np.arange(G)[:, None]) *
                        MASK_C).astype(ml_dtypes.bfloat16)
        mbw = np.full(W, -1, np.int64)
        mbw[:valid] = mb[lo:lo + valid]
        c16[:, NP:NP + W] = (
            mbw[None, :] == np.arange(G)[:, None]).astype(ml_dtypes.bfloat16)
        in_maps.append({"combo_x0": cx0, "combo_x1": cx1, "combo_w": cw,
                        "combo16": c16, "rows": rows_h})

    meta = {
        "los": los, "ms": ms, "me": me, "sorted_ok": sorted_ok,
        "ab": ab, "atom_x": atom_x, "motif_x": motif_x,
        "Wq": Wq, "bq": bq, "Wv": Wv, "bv": bv, "Wo": Wo, "bo": bo,
        "ln_g": ln_g, "ln_b": ln_b,
    }
    return nc, in_maps, meta


def unshard(results, meta):
    """Assemble per-core device results into the full (out, attn)."""
    los = meta["los"]
    out = np.empty((N_ATOMS, D), np.float32)
    attn = np.empty((N_ATOMS, H, N_MOTIFS), np.float32)
    for c in range(N_CORES):
        r = results[c]
        out[c * NP:(c + 1) * NP] = r["out"]
        a = r["attn_out"]  # [NP, H, 1536], window at cols [0, W)
        lo = los[c]
        dst = attn[c * NP:(c + 1) * NP]
        if lo == 0:
            dst[:] = a
        else:
            dst[:, :, lo:] = a[:, :, :N_MOTIFS - lo]
            dst[:, :, :lo] = a[:, :, N_MOTIFS - lo:]

    ln_g, ln_b = meta["ln_g"], meta["ln_b"]
    # device computes LN with g=1, b=0; apply g/b here if nontrivial
    if not (np.allclose(ln_g, 1.0) and np.allclose(ln_b, 0.0)):
        out = out * ln_g + ln_b

    # fallback for rows whose graph has no motifs (reference: uniform attn)
    if meta["sorted_ok"]:
        ms, me, ab = meta["ms"], meta["me"], meta["ab"]
        empty = np.where(me == ms)[0]
        if len(empty):
            bad = np.isin(ab, empty)
            if bad.any():
                idx = np.where(bad)[0]
                atom_x, motif_x = meta["atom_x"], meta["motif_x"]
                Wv, bv, Wo, bo = (meta[k] for k in ["Wv", "bv", "Wo", "bo"])
                attn[idx] = 1.0 / N_MOTIFS
                v_all = motif_x @ Wv.T + bv
                o = np.einsum("nhm,mhd->nhd",
                              attn[idx],
                              v_all.reshape(N_MOTIFS, H, HD)).reshape(
                                  len(idx), D)
                y = o @ Wo.T + bo + atom_x[idx]
                mu = y.mean(-1, keepdims=True)
                var = ((y - mu) ** 2).mean(-1, keepdims=True)
                out[idx] = (y - mu) / np.sqrt(var + LN_EPS) * \
                    meta["ln_g"] + meta["ln_b"]

    return out, attn


def _host_reference(atom_x, motif_x, atom_batch, motif_batch,
                    Wq, bq, Wk, bk, Wv, bv, Wo, bo, ln_g, ln_b):
    """Exact numpy fallback for pathological inputs (unsorted batches or
    window > 512) that the device path does not handle."""
    n = atom_x.shape[0]
    m = motif_x.shape[0]
    q = (atom_x @ Wq.T + bq).reshape(n, H, HD)
    k = (motif_x @ Wk.T + bk).reshape(m, H, HD)
    v = (motif_x @ Wv.T + bv).reshape(m, H, HD)
    scores = np.einsum("nhd,mhd->nhm", q, k) / np.sqrt(np.float32(HD))
    mask = atom_batch[:, None] == motif_batch[None, :]
    scores = np.where(mask[:, None, :], scores, np.float32(-1e9))
    scores -= scores.max(-1, keepdims=True)
    e = np.exp(scores)
    attn = (e / e.sum(-1, keepdims=True)).astype(np.float32)
    out = np.einsum("nhm,mhd->nhd", attn, v).reshape(n, H * HD)
    out = out @ Wo.T + bo + atom_x
    mu = out.mean(-1, keepdims=True)
    var = ((out - mu) ** 2).mean(-1, keepdims=True)
    out = ((out - mu) / np.sqrt(var + LN_EPS) * ln_g + ln_b).astype(
        np.float32)
    return out, attn


def kernel(**inputs):
    ab = np.asarray(inputs["atom_batch"]).astype(np.int64)
    mb = np.asarray(inputs["motif_batch"]).astype(np.int64)
    if np.all(np.diff(ab) >= 0) and np.all(np.diff(mb) >= 0):
        ms = np.searchsorted(mb, np.arange(128), "left")
        me = np.searchsorted(mb, np.arange(128), "right")
        span = max(
            int(me[ab[(c + 1) * NP - 1]]) - int(ms[ab[c * NP]])
            for c in range(N_CORES))
        device_ok = span <= 512
    else:
        device_ok = False
    if not device_ok:
        a = {k2: np.asarray(v2, dtype=np.float32) for k2, v2 in
             inputs.items() if k2 not in ("atom_batch", "motif_batch")}
        return _host_reference(atom_batch=ab, motif_batch=mb, **a)

    nc, in_maps, meta = prepare(**inputs)
    trace = bool(int(os.environ.get("BASS_KERNEL_TRACE", "0")))
    res = run_bass_kernel_spmd(nc, in_maps, core_ids=list(range(N_CORES)),
                               trace=trace)
    if trace and res.exec_time_ns is not None:
        print(f"HW exec time: {res.exec_time_ns} ns")
    return unshard(res.results, meta)
